# revision 11
# baseline (speedup 1.0000x reference)
"""Trainium2 Bass kernel for nn_CrossChannelAttention.

Reference computation (per batch b, pixel p, with C=128 channels, NUMS=16
groups of HEADS=8 channels, OUT=256):
    fm[g,p]  = relu(sum_h W1[g,h] * x[8g+h, p] + b1[g])          # [16, P]
    feat[(g,d), p] = fm[g,p] * x[d,p]                            # [2048, P]
    out[o,p] = sum_c W2[o,c] * feat[c,p] + b2[o]                 # [256, P]

Strategy: data-parallel over batch B=8 across the 8 NeuronCores (one batch
image per core, params replicated).  Per core:
  - prologue: fm via small matmuls (W1 scattered into a [128,16] lhsT) +
    relu (scalar engine); fm is round-tripped to DRAM (per 1024-pixel group)
    because DMA partition-broadcast needs a DRAM source.
  - fm row g is broadcast to 128 partitions in [128,1024] chunks, split
    between DRAM->SBUF broadcast DMAs (wide shapes fan out across all 16 DMA
    engines) and gpsimd.partition_broadcast, so no compute engine pays for
    replication.
  - feat = x * fm_rep on the vector engine as a pure-SBUF bf16 multiply
    (2x mode, ~430ns per [128,512]).
  - the PE runs only the 256 accumulating K=128 main matmuls (plus 8 fm
    matmuls), all bf16 N=512.
  Keeping the vector engine away from PSUM and the PE free of K-switches is
  what lets the PE run at its warm 2.4 GHz rate (a DVE op reading PSUM every
  iteration was measured to hold the PE at half clock for the whole kernel).
Accuracy: bf16 matmuls with fp32 PSUM accumulation; rel err ~4e-3.
"""

import numpy as np
import ml_dtypes

import concourse.bacc as bacc
import concourse.tile as tile
from concourse import mybir
from concourse.bass_utils import run_bass_kernel_spmd

F32 = mybir.dt.float32
BF16 = mybir.dt.bfloat16

B, C, H, W = 8, 128, 64, 64
NUMS, HEADS, OUT = 16, 8, 256
P = H * W          # 4096 pixels per image
PB = 512           # pixel block (one PSUM bank of fp32)
NPB = P // PB      # 8 pixel blocks
GRP = 1024         # broadcast chunk (2 pixel blocks)
NGRP = P // GRP    # 4 broadcast groups
N_CORES = 8
LOOKAHEAD = 5      # broadcast/feat pipeline depth (in g units) ahead of mains
GPSIMD_GS = {2, 5, 8, 11, 14}      # groups replicated via gpsimd.partition_broadcast
GPSIMD_TT_GS = {7, 15}             # groups whose feat-multiply runs on gpsimd

_CACHE = {}


def _build():
    nc = bacc.Bacc("TRN2", target_bir_lowering=False, debug=False,
                   num_devices=N_CORES)

    x_d = nc.dram_tensor("x", [C, P], F32, kind="ExternalInput")
    w1s_d = nc.dram_tensor("w1s", [C, NUMS], BF16, kind="ExternalInput")
    w2t_d = nc.dram_tensor("w2t", [C, NUMS * OUT], BF16, kind="ExternalInput")
    b1_d = nc.dram_tensor("b1c", [NUMS, 1], F32, kind="ExternalInput")
    b2_d = nc.dram_tensor("b2c", [C, 2], F32, kind="ExternalInput")
    out_d = nc.dram_tensor("out", [OUT, P], F32, kind="ExternalOutput")

    relu = mybir.ActivationFunctionType.Relu
    ident = mybir.ActivationFunctionType.Identity
    mult = mybir.AluOpType.mult

    with tile.TileContext(nc) as tc:
        with (
            tc.tile_pool(name="const", bufs=1) as cpool,
            tc.tile_pool(name="xp", bufs=2) as xp,
            tc.tile_pool(name="xbp", bufs=1) as xbp,
            tc.tile_pool(name="fmrow", bufs=1) as fmrowp,
            tc.tile_pool(name="repp", bufs=20) as repp,
            tc.tile_pool(name="feat", bufs=2 * LOOKAHEAD + 2) as featp,
            tc.tile_pool(name="osb", bufs=4) as osb,
            tc.tile_pool(name="ps", bufs=8, space="PSUM") as ps,
            tc.tile_pool(name="dr", bufs=4, space="DRAM") as drp,
        ):
            # ---- interleaved prologue + main loop ----
            # Emission order is chosen so that broadcast-group k's chain
            # (x -> fm -> DRAM -> broadcast -> feat) completes while the PE
            # is still busy with group k-1's main matmuls.
            xbs = [None] * NPB
            w1s_t = cpool.tile([C, NUMS], BF16)
            nc.sync.dma_start(w1s_t[:], w1s_d[:])
            b1_t = cpool.tile([NUMS, 1], F32)
            nc.sync.dma_start(b1_t[:], b1_d[:])

            fm_sb = cpool.tile([NUMS, P], BF16)
            fm_drs = [drp.tile([NUMS, GRP], BF16, tag=f"fmdr{k}",
                               name=f"fmdr{k}")
                      for k in range(NGRP)]
            fmrows = {}

            def emit_group_prologue(k):
                for pb in (2 * k, 2 * k + 1):
                    px = slice(pb * PB, (pb + 1) * PB)
                    x_t = xp.tile([C, PB], F32, tag="xt", name=f"xt{pb}")
                    nc.sync.dma_start(x_t[:], x_d[:, px])
                    x_b = xbp.tile([C, PB], BF16, tag=f"xb{pb}",
                                   name=f"xb{pb}")
                    nc.scalar.copy(x_b[:], x_t[:])
                    xbs[pb] = x_b
                    ps_fm = ps.tile([NUMS, PB], F32, tag="ps",
                                    name=f"psfm{pb}")
                    nc.tensor.matmul(ps_fm[:], w1s_t[:], x_b[:],
                                     start=True, stop=True)
                    nc.scalar.activation(fm_sb[:, px], ps_fm[:], relu,
                                         bias=b1_t[:])
                gx = slice(k * GRP, (k + 1) * GRP)
                nc.sync.dma_start(fm_drs[k][:], fm_sb[:, gx])
                for g in sorted(GPSIMD_GS):
                    fr = fmrowp.tile([1, GRP], BF16, tag=f"fr{g}_{k}",
                                     name=f"fr{g}_{k}")
                    nc.sync.dma_start(fr[:], fm_drs[k][g:g + 1, :])
                    fmrows[(g, k)] = fr

            def emit_rep_grp(g, k):
                rep = repp.tile([C, GRP], BF16, tag="rep", name=f"rep{g}_{k}")
                if g in GPSIMD_GS:
                    nc.gpsimd.partition_broadcast(rep[:], fmrows[(g, k)][0:1, :])
                else:
                    nc.sync.dma_start(
                        rep[:], fm_drs[k][g:g + 1, :].broadcast_to((C, GRP)))
                return rep

            fts = {}      # (g, k) -> [C, GRP] feat tile

            def emit_ft(g, k):
                rep = emit_rep_grp(g, k)
                ft = featp.tile([C, GRP], BF16, tag="ft", name=f"ft{g}_{k}")
                eng = nc.gpsimd if g in GPSIMD_TT_GS else nc.vector
                eng.tensor_tensor(ft[:], x2s[k][:], rep[:], op=mult)
                fts[(g, k)] = ft

            # pair the two pixel-blocks of each group into one bf16 tile so
            # the feat multiply runs as a single [128, 1024] op
            x2s = [None] * NGRP

            def emit_xpair(k):
                x2 = xbp.tile([C, GRP], BF16, tag=f"x2_{k}", name=f"x2_{k}")
                nc.vector.tensor_copy(x2[:, 0:PB], xbs[2 * k][:])
                nc.vector.tensor_copy(x2[:, PB:GRP], xbs[2 * k + 1][:])
                x2s[k] = x2

            emit_group_prologue(0)
            emit_xpair(0)
            w2t_t = cpool.tile([C, NUMS * OUT], BF16)
            nc.sync.dma_start(w2t_t[:], w2t_d[:])
            b2_t = cpool.tile([C, 2], F32)
            nc.sync.dma_start(b2_t[:], b2_d[:])
            emit_group_prologue(1)
            emit_xpair(1)

            todo = [(g, k) for k in range(NGRP) for g in range(NUMS)]
            for i in range(LOOKAHEAD):
                emit_ft(*todo[i])

            for i, (g, k) in enumerate(todo):
                if g == 0 and k + 2 < NGRP:
                    emit_group_prologue(k + 2)
                    emit_xpair(k + 2)
                if i + LOOKAHEAD < len(todo):
                    emit_ft(*todo[i + LOOKAHEAD])
                ft = fts.pop((g, k))
                for half in range(2):
                    pb = 2 * k + half
                    hx = slice(half * PB, (half + 1) * PB)
                    if g == 0 and half == 0:
                        pso = {}
                    if half == 0 and g == 0:
                        for pbb in (2 * k, 2 * k + 1):
                            for oc in range(2):
                                t = ps.tile([C, PB], F32, tag="ps",
                                            name=f"pso{pbb}_{oc}")
                                pso[(pbb, oc)] = t
                    nc.tensor.matmul(pso[(pb, 0)][:],
                                     w2t_t[:, (2 * g) * C:(2 * g + 1) * C],
                                     ft[:, hx], start=(g == 0),
                                     stop=(g == NUMS - 1))
                    nc.tensor.matmul(pso[(pb, 1)][:],
                                     w2t_t[:, (2 * g + 1) * C:(2 * g + 2) * C],
                                     ft[:, hx], start=(g == 0),
                                     stop=(g == NUMS - 1))
                if g == NUMS - 1:
                    for pbb in (2 * k, 2 * k + 1):
                        px = slice(pbb * PB, (pbb + 1) * PB)
                        o0 = osb.tile([C, PB], F32, tag="osb",
                                      name=f"o0_{pbb}")
                        o1 = osb.tile([C, PB], F32, tag="osb",
                                      name=f"o1_{pbb}")
                        nc.scalar.activation(o0[:], pso[(pbb, 0)][:], ident,
                                             bias=b2_t[:, 0:1])
                        nc.scalar.activation(o1[:], pso[(pbb, 1)][:], ident,
                                             bias=b2_t[:, 1:2])
                        nc.sync.dma_start(out_d[0:C, px], o0[:])
                        nc.sync.dma_start(out_d[C:OUT, px], o1[:])

    nc.compile()
    return nc


def _prep_params(W1, b1, W2, b2):
    bf = ml_dtypes.bfloat16
    # w1s[c, g] = W1[g, c - 8g] for 8g <= c < 8(g+1), else 0
    w1s = np.zeros((C, NUMS), dtype=bf)
    for g in range(NUMS):
        w1s[g * HEADS:(g + 1) * HEADS, g] = W1[g].astype(bf)
    # w2t[k, (g*2+oc)*128 + m] = W2[oc*128 + m, g*128 + k]
    w2t = (
        np.asarray(W2, dtype=np.float32)
        .reshape(2, C, NUMS, C)          # [oc, m, g, k]
        .transpose(3, 2, 0, 1)           # [k, g, oc, m]
        .reshape(C, NUMS * OUT)
        .astype(bf)
    )
    b1c = np.asarray(b1, dtype=np.float32).reshape(NUMS, 1).copy()
    b2c = np.asarray(b2, dtype=np.float32).reshape(2, C).T.copy()
    return w1s, w2t, b1c, b2c


def kernel(x, W1, b1, W2, b2, _trace=False, _trace_kwargs=None):
    if "nc" not in _CACHE:
        _CACHE["nc"] = _build()
    nc = _CACHE["nc"]

    w1s, w2t, b1c, b2c = _prep_params(W1, b1, W2, b2)
    xs = np.ascontiguousarray(np.asarray(x, dtype=np.float32).reshape(B, C, P))
    in_maps = [
        {"x": xs[b_], "w1s": w1s, "w2t": w2t, "b1c": b1c, "b2c": b2c}
        for b_ in range(N_CORES)
    ]
    kwargs = {}
    if _trace:
        kwargs["trace"] = True
        kwargs.update(_trace_kwargs or {})
    res = run_bass_kernel_spmd(nc, in_maps, core_ids=list(range(N_CORES)),
                               **kwargs)
    out = np.stack([res.results[b_]["out"] for b_ in range(N_CORES)])
    out = out.reshape(B, OUT, H, W)
    if _trace:
        _CACHE["last_result"] = res
    return out


# revision 12
# speedup vs baseline: 1.0486x; 1.0486x over previous
"""Trainium2 Bass kernel for nn_CrossChannelAttention.

Reference computation (per batch b, pixel p, with C=128 channels, NUMS=16
groups of HEADS=8 channels, OUT=256):
    fm[g,p]  = relu(sum_h W1[g,h] * x[8g+h, p] + b1[g])          # [16, P]
    feat[(g,d), p] = fm[g,p] * x[d,p]                            # [2048, P]
    out[o,p] = sum_c W2[o,c] * feat[c,p] + b2[o]                 # [256, P]

Strategy: data-parallel over batch B=8 across the 8 NeuronCores (one batch
image per core, params replicated).  Per core:
  - prologue: fm via small matmuls (W1 scattered into a [128,16] lhsT) +
    relu (scalar engine); fm is round-tripped to DRAM (per 1024-pixel group)
    because DMA partition-broadcast needs a DRAM source.
  - fm row g is broadcast to 128 partitions in [128,1024] chunks, split
    between DRAM->SBUF broadcast DMAs (wide shapes fan out across all 16 DMA
    engines) and gpsimd.partition_broadcast, so no compute engine pays for
    replication.
  - feat = x * fm_rep on the vector engine as a pure-SBUF bf16 multiply
    (2x mode, ~430ns per [128,512]).
  - the PE runs only the 256 accumulating K=128 main matmuls (plus 8 fm
    matmuls), all bf16 N=512.
  Keeping the vector engine away from PSUM and the PE free of K-switches is
  what lets the PE run at its warm 2.4 GHz rate (a DVE op reading PSUM every
  iteration was measured to hold the PE at half clock for the whole kernel).
Accuracy: bf16 matmuls with fp32 PSUM accumulation; rel err ~4e-3.
"""

import numpy as np
import ml_dtypes

import concourse.bacc as bacc
import concourse.tile as tile
from concourse import mybir
from concourse.bass_utils import run_bass_kernel_spmd

F32 = mybir.dt.float32
BF16 = mybir.dt.bfloat16

B, C, H, W = 8, 128, 64, 64
NUMS, HEADS, OUT = 16, 8, 256
P = H * W          # 4096 pixels per image
PB = 512           # pixel block (one PSUM bank of fp32)
NPB = P // PB      # 8 pixel blocks
GRP = 1024         # broadcast chunk (2 pixel blocks)
NGRP = P // GRP    # 4 broadcast groups
N_CORES = 8
LOOKAHEAD = 5      # broadcast/feat pipeline depth (in g units) ahead of mains
GPSIMD_GS = {2, 5, 8, 11, 14}      # groups replicated via gpsimd.partition_broadcast
GPSIMD_TT_GS = {7, 15}             # groups whose feat-multiply runs on gpsimd

_CACHE = {}


def _build():
    nc = bacc.Bacc("TRN2", target_bir_lowering=False, debug=False,
                   num_devices=N_CORES)

    x_d = nc.dram_tensor("x", [C, P], F32, kind="ExternalInput")
    w1s_d = nc.dram_tensor("w1s", [C, NUMS], BF16, kind="ExternalInput")
    w2t_d = nc.dram_tensor("w2t", [C, NUMS * OUT], BF16, kind="ExternalInput")
    b1_d = nc.dram_tensor("b1c", [NUMS, 1], F32, kind="ExternalInput")
    b2_d = nc.dram_tensor("b2c", [C, 2], F32, kind="ExternalInput")
    out_d = nc.dram_tensor("out", [OUT, P], F32, kind="ExternalOutput")

    relu = mybir.ActivationFunctionType.Relu
    ident = mybir.ActivationFunctionType.Identity
    mult = mybir.AluOpType.mult

    with tile.TileContext(nc) as tc:
        with (
            tc.tile_pool(name="const", bufs=1) as cpool,
            tc.tile_pool(name="xp", bufs=2) as xp,
            tc.tile_pool(name="xbp", bufs=1) as xbp,
            tc.tile_pool(name="fmrow", bufs=1) as fmrowp,
            tc.tile_pool(name="repp", bufs=20) as repp,
            tc.tile_pool(name="feat", bufs=2 * LOOKAHEAD + 2) as featp,
            tc.tile_pool(name="osb", bufs=4) as osb,
            tc.tile_pool(name="ps", bufs=8, space="PSUM") as ps,
            tc.tile_pool(name="dr", bufs=4, space="DRAM") as drp,
        ):
            # ---- prologue: x load/cast + fm, per broadcast group ----
            w1s_t = cpool.tile([C, NUMS], BF16)
            nc.sync.dma_start(w1s_t[:], w1s_d[:])
            b1_t = cpool.tile([NUMS, 1], F32)
            nc.sync.dma_start(b1_t[:], b1_d[:])

            fm_sb = cpool.tile([NUMS, P], BF16)
            fm_drs = [drp.tile([NUMS, GRP], BF16, tag=f"fmdr{k}",
                               name=f"fmdr{k}")
                      for k in range(NGRP)]
            fmrows = {}
            x2s = [None] * NGRP

            for k in range(NGRP):
                x2 = xbp.tile([C, GRP], BF16, tag=f"x2_{k}", name=f"x2_{k}")
                x2s[k] = x2
                for half in range(2):
                    pb = 2 * k + half
                    px = slice(pb * PB, (pb + 1) * PB)
                    hx = slice(half * PB, (half + 1) * PB)
                    x_t = xp.tile([C, PB], F32, tag="xt", name=f"xt{pb}")
                    nc.sync.dma_start(x_t[:], x_d[:, px])
                    nc.scalar.copy(x2[:, hx], x_t[:])
                    ps_fm = ps.tile([NUMS, PB], F32, tag="ps",
                                    name=f"psfm{pb}")
                    nc.tensor.matmul(ps_fm[:], w1s_t[:], x2[:, hx],
                                     start=True, stop=True)
                    nc.scalar.activation(fm_sb[:, px], ps_fm[:], relu,
                                         bias=b1_t[:])
                gx = slice(k * GRP, (k + 1) * GRP)
                nc.sync.dma_start(fm_drs[k][:], fm_sb[:, gx])
                for g in sorted(GPSIMD_GS):
                    fr = fmrowp.tile([1, GRP], BF16, tag=f"fr{g}_{k}",
                                     name=f"fr{g}_{k}")
                    nc.sync.dma_start(fr[:], fm_drs[k][g:g + 1, :])
                    fmrows[(g, k)] = fr
                if k == 0:
                    w2t_t = cpool.tile([C, NUMS * OUT], BF16)
                    nc.sync.dma_start(w2t_t[:], w2t_d[:])
                    b2_t = cpool.tile([C, 2], F32)
                    nc.sync.dma_start(b2_t[:], b2_d[:])

            # ---- replication + feat, pipelined ahead of the mains ----
            def emit_rep_grp(g, k):
                rep = repp.tile([C, GRP], BF16, tag="rep", name=f"rep{g}_{k}")
                if g in GPSIMD_GS:
                    nc.gpsimd.partition_broadcast(rep[:], fmrows[(g, k)][0:1, :])
                else:
                    nc.sync.dma_start(
                        rep[:], fm_drs[k][g:g + 1, :].broadcast_to((C, GRP)))
                return rep

            fts = {}      # (g, k) -> [C, GRP] feat tile

            def emit_ft(g, k):
                rep = emit_rep_grp(g, k)
                ft = featp.tile([C, GRP], BF16, tag="ft", name=f"ft{g}_{k}")
                eng = nc.gpsimd if g in GPSIMD_TT_GS else nc.vector
                eng.tensor_tensor(ft[:], x2s[k][:], rep[:], op=mult)
                fts[(g, k)] = ft

            todo = [(g, k) for k in range(NGRP) for g in range(NUMS)]
            for i in range(LOOKAHEAD):
                emit_ft(*todo[i])

            pso = {}
            for i, (g, k) in enumerate(todo):
                if i + LOOKAHEAD < len(todo):
                    emit_ft(*todo[i + LOOKAHEAD])
                ft = fts.pop((g, k))
                if g == 0:
                    for pbb in (2 * k, 2 * k + 1):
                        for oc in range(2):
                            t = ps.tile([C, PB], F32, tag="ps",
                                        name=f"pso{pbb}_{oc}")
                            pso[(pbb, oc)] = t
                for half in range(2):
                    pb = 2 * k + half
                    hx = slice(half * PB, (half + 1) * PB)
                    nc.tensor.matmul(pso[(pb, 0)][:],
                                     w2t_t[:, (2 * g) * C:(2 * g + 1) * C],
                                     ft[:, hx], start=(g == 0),
                                     stop=(g == NUMS - 1))
                    nc.tensor.matmul(pso[(pb, 1)][:],
                                     w2t_t[:, (2 * g + 1) * C:(2 * g + 2) * C],
                                     ft[:, hx], start=(g == 0),
                                     stop=(g == NUMS - 1))
                if g == NUMS - 1:
                    for pbb in (2 * k, 2 * k + 1):
                        px = slice(pbb * PB, (pbb + 1) * PB)
                        o0 = osb.tile([C, PB], F32, tag="osb",
                                      name=f"o0_{pbb}")
                        o1 = osb.tile([C, PB], F32, tag="osb",
                                      name=f"o1_{pbb}")
                        nc.scalar.activation(o0[:], pso.pop((pbb, 0))[:],
                                             ident, bias=b2_t[:, 0:1])
                        nc.scalar.activation(o1[:], pso.pop((pbb, 1))[:],
                                             ident, bias=b2_t[:, 1:2])
                        nc.sync.dma_start(out_d[0:C, px], o0[:])
                        nc.sync.dma_start(out_d[C:OUT, px], o1[:])

    nc.compile()
    return nc


def _prep_params(W1, b1, W2, b2):
    bf = ml_dtypes.bfloat16
    # w1s[c, g] = W1[g, c - 8g] for 8g <= c < 8(g+1), else 0
    w1s = np.zeros((C, NUMS), dtype=bf)
    for g in range(NUMS):
        w1s[g * HEADS:(g + 1) * HEADS, g] = W1[g].astype(bf)
    # w2t[k, (g*2+oc)*128 + m] = W2[oc*128 + m, g*128 + k]
    w2t = (
        np.asarray(W2, dtype=np.float32)
        .reshape(2, C, NUMS, C)          # [oc, m, g, k]
        .transpose(3, 2, 0, 1)           # [k, g, oc, m]
        .reshape(C, NUMS * OUT)
        .astype(bf)
    )
    b1c = np.asarray(b1, dtype=np.float32).reshape(NUMS, 1).copy()
    b2c = np.asarray(b2, dtype=np.float32).reshape(2, C).T.copy()
    return w1s, w2t, b1c, b2c


def kernel(x, W1, b1, W2, b2, _trace=False, _trace_kwargs=None):
    if "nc" not in _CACHE:
        _CACHE["nc"] = _build()
    nc = _CACHE["nc"]

    w1s, w2t, b1c, b2c = _prep_params(W1, b1, W2, b2)
    xs = np.ascontiguousarray(np.asarray(x, dtype=np.float32).reshape(B, C, P))
    in_maps = [
        {"x": xs[b_], "w1s": w1s, "w2t": w2t, "b1c": b1c, "b2c": b2c}
        for b_ in range(N_CORES)
    ]
    kwargs = {}
    if _trace:
        kwargs["trace"] = True
        kwargs.update(_trace_kwargs or {})
    res = run_bass_kernel_spmd(nc, in_maps, core_ids=list(range(N_CORES)),
                               **kwargs)
    out = np.stack([res.results[b_]["out"] for b_ in range(N_CORES)])
    out = out.reshape(B, OUT, H, W)
    if _trace:
        _CACHE["last_result"] = res
    return out


# revision 13
# speedup vs baseline: 1.8856x; 1.7982x over previous
"""Trainium2 Bass kernel for nn_CrossChannelAttention.

Reference computation (per batch b, pixel p, with C=128 channels, NUMS=16
groups of HEADS=8 channels, OUT=256):
    fm[g,p]  = relu(sum_h W1[g,h] * x[8g+h, p] + b1[g])          # [16, P]
    feat[(g,d), p] = fm[g,p] * x[d,p]                            # [2048, P]
    out[o,p] = sum_c W2[o,c] * feat[c,p] + b2[o]                 # [256, P]

Strategy: data-parallel over batch B=8 across the 8 NeuronCores (one batch
image per core, params replicated).  Per core:
  - prologue: fm via small matmuls (W1 scattered into a [128,16] lhsT) +
    relu (scalar engine); fm is round-tripped to DRAM (per 1024-pixel group)
    because DMA partition-broadcast needs a DRAM source.
  - fm row g is broadcast to 128 partitions in [128,1024] chunks, split
    between DRAM->SBUF broadcast DMAs (wide shapes fan out across all 16 DMA
    engines) and gpsimd.partition_broadcast, so no compute engine pays for
    replication.
  - feat = x * fm_rep on the vector engine as a pure-SBUF bf16 multiply
    (2x mode, ~430ns per [128,512]).
  - the PE runs only the 256 accumulating K=128 main matmuls (plus 8 fm
    matmuls), all bf16 N=512.
  Keeping the vector engine away from PSUM and the PE free of K-switches is
  what lets the PE run at its warm 2.4 GHz rate (a DVE op reading PSUM every
  iteration was measured to hold the PE at half clock for the whole kernel).
Accuracy: bf16 matmuls with fp32 PSUM accumulation; rel err ~4e-3.
"""

import numpy as np
import ml_dtypes

import concourse.bacc as bacc
import concourse.tile as tile
from concourse import mybir
from concourse.bass_utils import run_bass_kernel_spmd

F32 = mybir.dt.float32
BF16 = mybir.dt.bfloat16

B, C, H, W = 8, 128, 64, 64
NUMS, HEADS, OUT = 16, 8, 256
P = H * W          # 4096 pixels per image
PB = 512           # pixel block (one PSUM bank of fp32)
NPB = P // PB      # 8 pixel blocks
GRP = 1024         # broadcast chunk (2 pixel blocks)
NGRP = P // GRP    # 4 broadcast groups
N_CORES = 8
LOOKAHEAD = 5      # broadcast/feat pipeline depth (in g units) ahead of mains
GPSIMD_GS = {2, 5, 8, 11, 14}      # groups replicated via gpsimd.partition_broadcast
GPSIMD_TT_GS = set()               # keep gpsimd single-library: only partition_broadcast

_CACHE = {}


def _build():
    nc = bacc.Bacc("TRN2", target_bir_lowering=False, debug=False,
                   num_devices=N_CORES)

    x_d = nc.dram_tensor("x", [C, P], F32, kind="ExternalInput")
    w1s_d = nc.dram_tensor("w1s", [C, NUMS], BF16, kind="ExternalInput")
    w2t_d = nc.dram_tensor("w2t", [C, NUMS * OUT], BF16, kind="ExternalInput")
    b1_d = nc.dram_tensor("b1c", [NUMS, 1], F32, kind="ExternalInput")
    b2_d = nc.dram_tensor("b2c", [C, 2], F32, kind="ExternalInput")
    out_d = nc.dram_tensor("out", [OUT, P], F32, kind="ExternalOutput")

    relu = mybir.ActivationFunctionType.Relu
    ident = mybir.ActivationFunctionType.Identity
    mult = mybir.AluOpType.mult

    with tile.TileContext(nc) as tc:
        with (
            tc.tile_pool(name="const", bufs=1) as cpool,
            tc.tile_pool(name="xp", bufs=2) as xp,
            tc.tile_pool(name="xbp", bufs=1) as xbp,
            tc.tile_pool(name="fmrow", bufs=1) as fmrowp,
            tc.tile_pool(name="repp", bufs=20) as repp,
            tc.tile_pool(name="feat", bufs=2 * LOOKAHEAD + 2) as featp,
            tc.tile_pool(name="osb", bufs=4) as osb,
            tc.tile_pool(name="ps", bufs=8, space="PSUM") as ps,
            tc.tile_pool(name="dr", bufs=4, space="DRAM") as drp,
        ):
            # ---- prologue: x load/cast + fm, per broadcast group ----
            w1s_t = cpool.tile([C, NUMS], BF16)
            nc.sync.dma_start(w1s_t[:], w1s_d[:])
            b1_t = cpool.tile([NUMS, 1], F32)
            nc.sync.dma_start(b1_t[:], b1_d[:])

            fm_sb = cpool.tile([NUMS, P], BF16)
            fm_drs = [drp.tile([NUMS, GRP], BF16, tag=f"fmdr{k}",
                               name=f"fmdr{k}")
                      for k in range(NGRP)]
            fmrows = {}
            x2s = [None] * NGRP

            for k in range(NGRP):
                x2 = xbp.tile([C, GRP], BF16, tag=f"x2_{k}", name=f"x2_{k}")
                x2s[k] = x2
                for half in range(2):
                    pb = 2 * k + half
                    px = slice(pb * PB, (pb + 1) * PB)
                    hx = slice(half * PB, (half + 1) * PB)
                    x_t = xp.tile([C, PB], F32, tag="xt", name=f"xt{pb}")
                    nc.sync.dma_start(x_t[:], x_d[:, px])
                    nc.scalar.copy(x2[:, hx], x_t[:])
                    ps_fm = ps.tile([NUMS, PB], F32, tag="ps",
                                    name=f"psfm{pb}")
                    nc.tensor.matmul(ps_fm[:], w1s_t[:], x2[:, hx],
                                     start=True, stop=True)
                    nc.scalar.activation(fm_sb[:, px], ps_fm[:], relu,
                                         bias=b1_t[:])
                gx = slice(k * GRP, (k + 1) * GRP)
                nc.sync.dma_start(fm_drs[k][:], fm_sb[:, gx])
                for g in sorted(GPSIMD_GS):
                    fr = fmrowp.tile([1, GRP], BF16, tag=f"fr{g}_{k}",
                                     name=f"fr{g}_{k}")
                    nc.sync.dma_start(fr[:], fm_drs[k][g:g + 1, :])
                    fmrows[(g, k)] = fr
                if k == 0:
                    w2t_t = cpool.tile([C, NUMS * OUT], BF16)
                    nc.sync.dma_start(w2t_t[:], w2t_d[:])
                    b2_t = cpool.tile([C, 2], F32)
                    nc.sync.dma_start(b2_t[:], b2_d[:])

            # ---- replication + feat, pipelined ahead of the mains ----
            def emit_rep_grp(g, k):
                rep = repp.tile([C, GRP], BF16, tag="rep", name=f"rep{g}_{k}")
                if g in GPSIMD_GS:
                    nc.gpsimd.partition_broadcast(rep[:], fmrows[(g, k)][0:1, :])
                else:
                    nc.sync.dma_start(
                        rep[:], fm_drs[k][g:g + 1, :].broadcast_to((C, GRP)))
                return rep

            fts = {}      # (g, k) -> [C, GRP] feat tile

            def emit_ft(g, k):
                rep = emit_rep_grp(g, k)
                ft = featp.tile([C, GRP], BF16, tag="ft", name=f"ft{g}_{k}")
                eng = nc.gpsimd if g in GPSIMD_TT_GS else nc.vector
                eng.tensor_tensor(ft[:], x2s[k][:], rep[:], op=mult)
                fts[(g, k)] = ft

            todo = [(g, k) for k in range(NGRP) for g in range(NUMS)]
            for i in range(LOOKAHEAD):
                emit_ft(*todo[i])

            pso = {}
            for i, (g, k) in enumerate(todo):
                if i + LOOKAHEAD < len(todo):
                    emit_ft(*todo[i + LOOKAHEAD])
                ft = fts.pop((g, k))
                if g == 0:
                    for pbb in (2 * k, 2 * k + 1):
                        for oc in range(2):
                            t = ps.tile([C, PB], F32, tag="ps",
                                        name=f"pso{pbb}_{oc}")
                            pso[(pbb, oc)] = t
                for half in range(2):
                    pb = 2 * k + half
                    hx = slice(half * PB, (half + 1) * PB)
                    nc.tensor.matmul(pso[(pb, 0)][:],
                                     w2t_t[:, (2 * g) * C:(2 * g + 1) * C],
                                     ft[:, hx], start=(g == 0),
                                     stop=(g == NUMS - 1))
                    nc.tensor.matmul(pso[(pb, 1)][:],
                                     w2t_t[:, (2 * g + 1) * C:(2 * g + 2) * C],
                                     ft[:, hx], start=(g == 0),
                                     stop=(g == NUMS - 1))
                if g == NUMS - 1:
                    for pbb in (2 * k, 2 * k + 1):
                        px = slice(pbb * PB, (pbb + 1) * PB)
                        o0 = osb.tile([C, PB], F32, tag="osb",
                                      name=f"o0_{pbb}")
                        o1 = osb.tile([C, PB], F32, tag="osb",
                                      name=f"o1_{pbb}")
                        nc.scalar.activation(o0[:], pso.pop((pbb, 0))[:],
                                             ident, bias=b2_t[:, 0:1])
                        nc.scalar.activation(o1[:], pso.pop((pbb, 1))[:],
                                             ident, bias=b2_t[:, 1:2])
                        nc.sync.dma_start(out_d[0:C, px], o0[:])
                        nc.sync.dma_start(out_d[C:OUT, px], o1[:])

    nc.compile()
    return nc


def _prep_params(W1, b1, W2, b2):
    bf = ml_dtypes.bfloat16
    # w1s[c, g] = W1[g, c - 8g] for 8g <= c < 8(g+1), else 0
    w1s = np.zeros((C, NUMS), dtype=bf)
    for g in range(NUMS):
        w1s[g * HEADS:(g + 1) * HEADS, g] = W1[g].astype(bf)
    # w2t[k, (g*2+oc)*128 + m] = W2[oc*128 + m, g*128 + k]
    w2t = (
        np.asarray(W2, dtype=np.float32)
        .reshape(2, C, NUMS, C)          # [oc, m, g, k]
        .transpose(3, 2, 0, 1)           # [k, g, oc, m]
        .reshape(C, NUMS * OUT)
        .astype(bf)
    )
    b1c = np.asarray(b1, dtype=np.float32).reshape(NUMS, 1).copy()
    b2c = np.asarray(b2, dtype=np.float32).reshape(2, C).T.copy()
    return w1s, w2t, b1c, b2c


def kernel(x, W1, b1, W2, b2, _trace=False, _trace_kwargs=None):
    if "nc" not in _CACHE:
        _CACHE["nc"] = _build()
    nc = _CACHE["nc"]

    w1s, w2t, b1c, b2c = _prep_params(W1, b1, W2, b2)
    xs = np.ascontiguousarray(np.asarray(x, dtype=np.float32).reshape(B, C, P))
    in_maps = [
        {"x": xs[b_], "w1s": w1s, "w2t": w2t, "b1c": b1c, "b2c": b2c}
        for b_ in range(N_CORES)
    ]
    kwargs = {}
    if _trace:
        kwargs["trace"] = True
        kwargs.update(_trace_kwargs or {})
    res = run_bass_kernel_spmd(nc, in_maps, core_ids=list(range(N_CORES)),
                               **kwargs)
    out = np.stack([res.results[b_]["out"] for b_ in range(N_CORES)])
    out = out.reshape(B, OUT, H, W)
    if _trace:
        _CACHE["last_result"] = res
    return out


# revision 14
# speedup vs baseline: 1.9794x; 1.0498x over previous
"""Trainium2 Bass kernel for nn_CrossChannelAttention.

Reference computation (per batch b, pixel p, with C=128 channels, NUMS=16
groups of HEADS=8 channels, OUT=256):
    fm[g,p]  = relu(sum_h W1[g,h] * x[8g+h, p] + b1[g])          # [16, P]
    feat[(g,d), p] = fm[g,p] * x[d,p]                            # [2048, P]
    out[o,p] = sum_c W2[o,c] * feat[c,p] + b2[o]                 # [256, P]

Strategy: data-parallel over batch B=8 across the 8 NeuronCores (one batch
image per core, params replicated).  Per core:
  - prologue: fm via small matmuls (W1 scattered into a [128,16] lhsT) +
    relu (scalar engine); fm is round-tripped to DRAM (per 1024-pixel group)
    because DMA partition-broadcast needs a DRAM source.
  - fm row g is broadcast to 128 partitions in [128,1024] chunks, split
    between DRAM->SBUF broadcast DMAs (wide shapes fan out across all 16 DMA
    engines) and gpsimd.partition_broadcast, so no compute engine pays for
    replication.
  - feat = x * fm_rep on the vector engine as a pure-SBUF bf16 multiply
    (2x mode, ~430ns per [128,512]).
  - the PE runs only the 256 accumulating K=128 main matmuls (plus 8 fm
    matmuls), all bf16 N=512.
  Keeping the vector engine away from PSUM and the PE free of K-switches is
  what lets the PE run at its warm 2.4 GHz rate (a DVE op reading PSUM every
  iteration was measured to hold the PE at half clock for the whole kernel).
Accuracy: bf16 matmuls with fp32 PSUM accumulation; rel err ~4e-3.
"""

import numpy as np
import ml_dtypes

import concourse.bacc as bacc
import concourse.tile as tile
from concourse import mybir
from concourse.bass_utils import run_bass_kernel_spmd

F32 = mybir.dt.float32
BF16 = mybir.dt.bfloat16

B, C, H, W = 8, 128, 64, 64
NUMS, HEADS, OUT = 16, 8, 256
P = H * W          # 4096 pixels per image
PB = 512           # pixel block (one PSUM bank of fp32)
NPB = P // PB      # 8 pixel blocks
GRP = 1024         # broadcast chunk (2 pixel blocks)
NGRP = P // GRP    # 4 broadcast groups
N_CORES = 8
LOOKAHEAD = 8      # broadcast/feat pipeline depth (in (g,k) units) ahead of mains
GPSIMD_GS = {2, 5, 8, 11, 14}      # groups replicated via gpsimd.partition_broadcast
GPSIMD_TT_GS = set()               # keep gpsimd single-library: only partition_broadcast

_CACHE = {}


def _build():
    nc = bacc.Bacc("TRN2", target_bir_lowering=False, debug=False,
                   num_devices=N_CORES)

    x_d = nc.dram_tensor("x", [C, P], F32, kind="ExternalInput")
    w1s_d = nc.dram_tensor("w1s", [C, NUMS], BF16, kind="ExternalInput")
    w2t_d = nc.dram_tensor("w2t", [C, NUMS * OUT], BF16, kind="ExternalInput")
    b1_d = nc.dram_tensor("b1c", [NUMS, 1], F32, kind="ExternalInput")
    b2_d = nc.dram_tensor("b2c", [C, 2], F32, kind="ExternalInput")
    out_d = nc.dram_tensor("out", [OUT, P], F32, kind="ExternalOutput")

    relu = mybir.ActivationFunctionType.Relu
    ident = mybir.ActivationFunctionType.Identity
    mult = mybir.AluOpType.mult

    with tile.TileContext(nc) as tc:
        with (
            tc.tile_pool(name="const", bufs=1) as cpool,
            tc.tile_pool(name="xp", bufs=2) as xp,
            tc.tile_pool(name="xbp", bufs=1) as xbp,
            tc.tile_pool(name="fmrow", bufs=1) as fmrowp,
            tc.tile_pool(name="repp", bufs=20) as repp,
            tc.tile_pool(name="feat", bufs=2 * LOOKAHEAD + 2) as featp,
            tc.tile_pool(name="osb", bufs=4) as osb,
            tc.tile_pool(name="ps", bufs=8, space="PSUM") as ps,
            tc.tile_pool(name="dr", bufs=4, space="DRAM") as drp,
        ):
            # ---- prologue: x load/cast + fm, per broadcast group ----
            w1s_t = cpool.tile([C, NUMS], BF16)
            nc.sync.dma_start(w1s_t[:], w1s_d[:])
            b1_t = cpool.tile([NUMS, 1], F32)
            nc.sync.dma_start(b1_t[:], b1_d[:])

            fm_sb = cpool.tile([NUMS, P], BF16)
            fm_drs = [drp.tile([NUMS, GRP], BF16, tag=f"fmdr{k}",
                               name=f"fmdr{k}")
                      for k in range(NGRP)]
            fmrows = {}
            x2s = [None] * NGRP

            def emit_fm_write(k):
                gx = slice(k * GRP, (k + 1) * GRP)
                nc.sync.dma_start(fm_drs[k][:], fm_sb[:, gx])
                for g in sorted(GPSIMD_GS):
                    fr = fmrowp.tile([1, GRP], BF16, tag=f"fr{g}_{k}",
                                     name=f"fr{g}_{k}")
                    nc.scalar.dma_start(fr[:], fm_drs[k][g:g + 1, :])
                    fmrows[(g, k)] = fr

            for k in range(NGRP):
                x2 = xbp.tile([C, GRP], BF16, tag=f"x2_{k}", name=f"x2_{k}")
                x2s[k] = x2
                for half in range(2):
                    pb = 2 * k + half
                    px = slice(pb * PB, (pb + 1) * PB)
                    hx = slice(half * PB, (half + 1) * PB)
                    x_t = xp.tile([C, PB], F32, tag="xt", name=f"xt{pb}")
                    nc.sync.dma_start(x_t[:], x_d[:, px])
                    nc.scalar.copy(x2[:, hx], x_t[:])
                    ps_fm = ps.tile([NUMS, PB], F32, tag="ps",
                                    name=f"psfm{pb}")
                    nc.tensor.matmul(ps_fm[:], w1s_t[:], x2[:, hx],
                                     start=True, stop=True)
                    nc.scalar.activation(fm_sb[:, px], ps_fm[:], relu,
                                         bias=b1_t[:])
                if k == 0:
                    emit_fm_write(0)
                    w2t_t = cpool.tile([C, NUMS * OUT], BF16)
                    nc.sync.dma_start(w2t_t[:], w2t_d[:])
                    b2_t = cpool.tile([C, 2], F32)
                    nc.sync.dma_start(b2_t[:], b2_d[:])

            # ---- replication + feat, pipelined ahead of the mains ----
            def emit_rep_grp(g, k):
                rep = repp.tile([C, GRP], BF16, tag="rep", name=f"rep{g}_{k}")
                if g in GPSIMD_GS:
                    nc.gpsimd.partition_broadcast(rep[:], fmrows[(g, k)][0:1, :])
                else:
                    nc.sync.dma_start(
                        rep[:], fm_drs[k][g:g + 1, :].broadcast_to((C, GRP)))
                return rep

            fts = {}      # (g, k) -> [C, GRP] feat tile

            def emit_ft(g, k):
                rep = emit_rep_grp(g, k)
                ft = featp.tile([C, GRP], BF16, tag="ft", name=f"ft{g}_{k}")
                eng = nc.gpsimd if g in GPSIMD_TT_GS else nc.vector
                eng.tensor_tensor(ft[:], x2s[k][:], rep[:], op=mult)
                fts[(g, k)] = ft

            todo = [(g, k) for k in range(NGRP) for g in range(NUMS)]
            for i in range(LOOKAHEAD):
                emit_ft(*todo[i])

            pso = {}
            for i, (g, k) in enumerate(todo):
                if g == 0 and k + 1 < NGRP:
                    emit_fm_write(k + 1)
                if i + LOOKAHEAD < len(todo):
                    emit_ft(*todo[i + LOOKAHEAD])
                ft = fts.pop((g, k))
                if g == 0:
                    for pbb in (2 * k, 2 * k + 1):
                        for oc in range(2):
                            t = ps.tile([C, PB], F32, tag="ps",
                                        name=f"pso{pbb}_{oc}")
                            pso[(pbb, oc)] = t
                for half in range(2):
                    pb = 2 * k + half
                    hx = slice(half * PB, (half + 1) * PB)
                    nc.tensor.matmul(pso[(pb, 0)][:],
                                     w2t_t[:, (2 * g) * C:(2 * g + 1) * C],
                                     ft[:, hx], start=(g == 0),
                                     stop=(g == NUMS - 1))
                    nc.tensor.matmul(pso[(pb, 1)][:],
                                     w2t_t[:, (2 * g + 1) * C:(2 * g + 2) * C],
                                     ft[:, hx], start=(g == 0),
                                     stop=(g == NUMS - 1))
                if g == NUMS - 1:
                    for pbb in (2 * k, 2 * k + 1):
                        px = slice(pbb * PB, (pbb + 1) * PB)
                        o0 = osb.tile([C, PB], F32, tag="osb",
                                      name=f"o0_{pbb}")
                        o1 = osb.tile([C, PB], F32, tag="osb",
                                      name=f"o1_{pbb}")
                        nc.scalar.activation(o0[:], pso.pop((pbb, 0))[:],
                                             ident, bias=b2_t[:, 0:1])
                        nc.scalar.activation(o1[:], pso.pop((pbb, 1))[:],
                                             ident, bias=b2_t[:, 1:2])
                        nc.scalar.dma_start(out_d[0:C, px], o0[:])
                        nc.scalar.dma_start(out_d[C:OUT, px], o1[:])

    nc.compile()
    return nc


def _prep_params(W1, b1, W2, b2):
    bf = ml_dtypes.bfloat16
    # w1s[c, g] = W1[g, c - 8g] for 8g <= c < 8(g+1), else 0
    w1s = np.zeros((C, NUMS), dtype=bf)
    for g in range(NUMS):
        w1s[g * HEADS:(g + 1) * HEADS, g] = W1[g].astype(bf)
    # w2t[k, (g*2+oc)*128 + m] = W2[oc*128 + m, g*128 + k]
    w2t = (
        np.asarray(W2, dtype=np.float32)
        .reshape(2, C, NUMS, C)          # [oc, m, g, k]
        .transpose(3, 2, 0, 1)           # [k, g, oc, m]
        .reshape(C, NUMS * OUT)
        .astype(bf)
    )
    b1c = np.asarray(b1, dtype=np.float32).reshape(NUMS, 1).copy()
    b2c = np.asarray(b2, dtype=np.float32).reshape(2, C).T.copy()
    return w1s, w2t, b1c, b2c


def kernel(x, W1, b1, W2, b2, _trace=False, _trace_kwargs=None):
    if "nc" not in _CACHE:
        _CACHE["nc"] = _build()
    nc = _CACHE["nc"]

    w1s, w2t, b1c, b2c = _prep_params(W1, b1, W2, b2)
    xs = np.ascontiguousarray(np.asarray(x, dtype=np.float32).reshape(B, C, P))
    in_maps = [
        {"x": xs[b_], "w1s": w1s, "w2t": w2t, "b1c": b1c, "b2c": b2c}
        for b_ in range(N_CORES)
    ]
    kwargs = {}
    if _trace:
        kwargs["trace"] = True
        kwargs.update(_trace_kwargs or {})
    res = run_bass_kernel_spmd(nc, in_maps, core_ids=list(range(N_CORES)),
                               **kwargs)
    out = np.stack([res.results[b_]["out"] for b_ in range(N_CORES)])
    out = out.reshape(B, OUT, H, W)
    if _trace:
        _CACHE["last_result"] = res
    return out


# revision 15
# speedup vs baseline: 2.0157x; 1.0183x over previous
"""Trainium2 Bass kernel for nn_CrossChannelAttention.

Reference computation (per batch b, pixel p, with C=128 channels, NUMS=16
groups of HEADS=8 channels, OUT=256):
    fm[g,p]  = relu(sum_h W1[g,h] * x[8g+h, p] + b1[g])          # [16, P]
    feat[(g,d), p] = fm[g,p] * x[d,p]                            # [2048, P]
    out[o,p] = sum_c W2[o,c] * feat[c,p] + b2[o]                 # [256, P]

Strategy: data-parallel over batch B=8 across the 8 NeuronCores (one batch
image per core, params replicated).  Per core:
  - prologue: fm via small matmuls (W1 scattered into a [128,16] lhsT) +
    relu (scalar engine); fm is round-tripped to DRAM (per 1024-pixel group)
    because DMA partition-broadcast needs a DRAM source.
  - fm row g is broadcast to 128 partitions in [128,1024] chunks, split
    between DRAM->SBUF broadcast DMAs (wide shapes fan out across all 16 DMA
    engines) and gpsimd.partition_broadcast, so no compute engine pays for
    replication.
  - feat = x * fm_rep on the vector engine as a pure-SBUF bf16 multiply
    (2x mode, ~430ns per [128,512]).
  - the PE runs only the 256 accumulating K=128 main matmuls (plus 8 fm
    matmuls), all bf16 N=512.
  Keeping the vector engine away from PSUM and the PE free of K-switches is
  what lets the PE run at its warm 2.4 GHz rate (a DVE op reading PSUM every
  iteration was measured to hold the PE at half clock for the whole kernel).
Accuracy: bf16 matmuls with fp32 PSUM accumulation; rel err ~4e-3.
"""

import numpy as np
import ml_dtypes

import concourse.bacc as bacc
import concourse.tile as tile
from concourse import mybir
from concourse.bass_utils import run_bass_kernel_spmd

F32 = mybir.dt.float32
BF16 = mybir.dt.bfloat16

B, C, H, W = 8, 128, 64, 64
NUMS, HEADS, OUT = 16, 8, 256
P = H * W          # 4096 pixels per image
PB = 512           # pixel block (one PSUM bank of fp32)
NPB = P // PB      # 8 pixel blocks
GRP = 1024         # broadcast chunk (2 pixel blocks)
NGRP = P // GRP    # 4 broadcast groups
N_CORES = 8
LOOKAHEAD = 8      # broadcast/feat pipeline depth (in (g,k) units) ahead of mains
GPSIMD_TT = {2, 5, 8, 11, 14}      # groups whose feat-multiply runs on gpsimd

_CACHE = {}


def _build():
    nc = bacc.Bacc("TRN2", target_bir_lowering=False, debug=False,
                   num_devices=N_CORES)

    x_d = nc.dram_tensor("x", [C, P], BF16, kind="ExternalInput")
    w1s_d = nc.dram_tensor("w1s", [C, NUMS], BF16, kind="ExternalInput")
    w2t_d = nc.dram_tensor("w2t", [C, NUMS * OUT], BF16, kind="ExternalInput")
    b1_d = nc.dram_tensor("b1c", [NUMS, 1], F32, kind="ExternalInput")
    b2_d = nc.dram_tensor("b2c", [C, 2], F32, kind="ExternalInput")
    out_d = nc.dram_tensor("out", [OUT, P], F32, kind="ExternalOutput")

    relu = mybir.ActivationFunctionType.Relu
    ident = mybir.ActivationFunctionType.Identity
    mult = mybir.AluOpType.mult

    with tile.TileContext(nc) as tc:
        with (
            tc.tile_pool(name="const", bufs=1) as cpool,
            tc.tile_pool(name="xp", bufs=2) as xp,
            tc.tile_pool(name="xbp", bufs=1) as xbp,
            tc.tile_pool(name="fmrow", bufs=1) as fmrowp,
            tc.tile_pool(name="repp", bufs=20) as repp,
            tc.tile_pool(name="feat", bufs=2 * LOOKAHEAD + 2) as featp,
            tc.tile_pool(name="osb", bufs=4) as osb,
            tc.tile_pool(name="ps", bufs=8, space="PSUM") as ps,
            tc.tile_pool(name="dr", bufs=4, space="DRAM") as drp,
        ):
            # ---- prologue: x load/cast + fm, per broadcast group ----
            w1s_t = cpool.tile([C, NUMS], BF16)
            nc.sync.dma_start(w1s_t[:], w1s_d[:])
            b1_t = cpool.tile([NUMS, 1], F32)
            nc.sync.dma_start(b1_t[:], b1_d[:])

            fm_sb = cpool.tile([NUMS, P], BF16)
            fm_drs = [drp.tile([NUMS, GRP], BF16, tag=f"fmdr{k}",
                               name=f"fmdr{k}")
                      for k in range(NGRP)]
            fmrows = {}
            x2s = [None] * NGRP

            def emit_fm_write(k):
                gx = slice(k * GRP, (k + 1) * GRP)
                nc.sync.dma_start(fm_drs[k][:], fm_sb[:, gx])

            for k in range(NGRP):
                x2 = xbp.tile([C, GRP], BF16, tag=f"x2_{k}", name=f"x2_{k}")
                x2s[k] = x2
                gx = slice(k * GRP, (k + 1) * GRP)
                nc.sync.dma_start(x2[:], x_d[:, gx])
                for half in range(2):
                    pb = 2 * k + half
                    px = slice(pb * PB, (pb + 1) * PB)
                    hx = slice(half * PB, (half + 1) * PB)
                    ps_fm = ps.tile([NUMS, PB], F32, tag="ps",
                                    name=f"psfm{pb}")
                    nc.tensor.matmul(ps_fm[:], w1s_t[:], x2[:, hx],
                                     start=True, stop=True)
                    nc.scalar.activation(fm_sb[:, px], ps_fm[:], relu,
                                         bias=b1_t[:])
                if k == 0:
                    emit_fm_write(0)
                    w2t_t = cpool.tile([C, NUMS * OUT], BF16)
                    nc.sync.dma_start(w2t_t[:], w2t_d[:])
                    b2_t = cpool.tile([C, 2], F32)
                    nc.sync.dma_start(b2_t[:], b2_d[:])

            # ---- replication + feat, pipelined ahead of the mains ----
            nbc = [0]

            def emit_rep_grp(g, k):
                rep = repp.tile([C, GRP], BF16, tag="rep", name=f"rep{g}_{k}")
                eng = nc.sync if nbc[0] % 2 == 0 else nc.scalar
                nbc[0] += 1
                eng.dma_start(rep[:],
                              fm_drs[k][g:g + 1, :].broadcast_to((C, GRP)))
                return rep

            fts = {}      # (g, k) -> [C, GRP] feat tile

            def emit_ft(g, k):
                rep = emit_rep_grp(g, k)
                ft = featp.tile([C, GRP], BF16, tag="ft", name=f"ft{g}_{k}")
                eng = nc.gpsimd if g in GPSIMD_TT else nc.vector
                eng.tensor_tensor(ft[:], x2s[k][:], rep[:], op=mult)
                fts[(g, k)] = ft

            todo = [(g, k) for k in range(NGRP) for g in range(NUMS)]
            for i in range(LOOKAHEAD):
                emit_ft(*todo[i])

            pso = {}
            for i, (g, k) in enumerate(todo):
                if g == 0 and k + 1 < NGRP:
                    emit_fm_write(k + 1)
                if i + LOOKAHEAD < len(todo):
                    emit_ft(*todo[i + LOOKAHEAD])
                ft = fts.pop((g, k))
                if g == 0:
                    for pbb in (2 * k, 2 * k + 1):
                        for oc in range(2):
                            t = ps.tile([C, PB], F32, tag="ps",
                                        name=f"pso{pbb}_{oc}")
                            pso[(pbb, oc)] = t
                for half in range(2):
                    pb = 2 * k + half
                    hx = slice(half * PB, (half + 1) * PB)
                    nc.tensor.matmul(pso[(pb, 0)][:],
                                     w2t_t[:, (2 * g) * C:(2 * g + 1) * C],
                                     ft[:, hx], start=(g == 0),
                                     stop=(g == NUMS - 1))
                    nc.tensor.matmul(pso[(pb, 1)][:],
                                     w2t_t[:, (2 * g + 1) * C:(2 * g + 2) * C],
                                     ft[:, hx], start=(g == 0),
                                     stop=(g == NUMS - 1))
                if g == NUMS - 1:
                    for pbb in (2 * k, 2 * k + 1):
                        px = slice(pbb * PB, (pbb + 1) * PB)
                        o0 = osb.tile([C, PB], F32, tag="osb",
                                      name=f"o0_{pbb}")
                        o1 = osb.tile([C, PB], F32, tag="osb",
                                      name=f"o1_{pbb}")
                        nc.scalar.activation(o0[:], pso.pop((pbb, 0))[:],
                                             ident, bias=b2_t[:, 0:1])
                        nc.scalar.activation(o1[:], pso.pop((pbb, 1))[:],
                                             ident, bias=b2_t[:, 1:2])
                        nc.scalar.dma_start(out_d[0:C, px], o0[:])
                        nc.scalar.dma_start(out_d[C:OUT, px], o1[:])

    nc.compile()
    return nc


def _prep_params(W1, b1, W2, b2):
    bf = ml_dtypes.bfloat16
    # w1s[c, g] = W1[g, c - 8g] for 8g <= c < 8(g+1), else 0
    w1s = np.zeros((C, NUMS), dtype=bf)
    for g in range(NUMS):
        w1s[g * HEADS:(g + 1) * HEADS, g] = W1[g].astype(bf)
    # w2t[k, (g*2+oc)*128 + m] = W2[oc*128 + m, g*128 + k]
    w2t = (
        np.asarray(W2, dtype=np.float32)
        .reshape(2, C, NUMS, C)          # [oc, m, g, k]
        .transpose(3, 2, 0, 1)           # [k, g, oc, m]
        .reshape(C, NUMS * OUT)
        .astype(bf)
    )
    b1c = np.asarray(b1, dtype=np.float32).reshape(NUMS, 1).copy()
    b2c = np.asarray(b2, dtype=np.float32).reshape(2, C).T.copy()
    return w1s, w2t, b1c, b2c


def kernel(x, W1, b1, W2, b2, _trace=False, _trace_kwargs=None):
    if "nc" not in _CACHE:
        _CACHE["nc"] = _build()
    nc = _CACHE["nc"]

    w1s, w2t, b1c, b2c = _prep_params(W1, b1, W2, b2)
    xs = np.ascontiguousarray(
        np.asarray(x, dtype=np.float32).reshape(B, C, P).astype(ml_dtypes.bfloat16))
    in_maps = [
        {"x": xs[b_], "w1s": w1s, "w2t": w2t, "b1c": b1c, "b2c": b2c}
        for b_ in range(N_CORES)
    ]
    kwargs = {}
    if _trace:
        kwargs["trace"] = True
        kwargs.update(_trace_kwargs or {})
    res = run_bass_kernel_spmd(nc, in_maps, core_ids=list(range(N_CORES)),
                               **kwargs)
    out = np.stack([res.results[b_]["out"] for b_ in range(N_CORES)])
    out = out.reshape(B, OUT, H, W)
    if _trace:
        _CACHE["last_result"] = res
    return out


# revision 16
# speedup vs baseline: 2.0217x; 1.0029x over previous
"""Trainium2 Bass kernel for nn_CrossChannelAttention.

Reference computation (per batch b, pixel p, with C=128 channels, NUMS=16
groups of HEADS=8 channels, OUT=256):
    fm[g,p]  = relu(sum_h W1[g,h] * x[8g+h, p] + b1[g])          # [16, P]
    feat[(g,d), p] = fm[g,p] * x[d,p]                            # [2048, P]
    out[o,p] = sum_c W2[o,c] * feat[c,p] + b2[o]                 # [256, P]

Strategy: data-parallel over batch B=8 across the 8 NeuronCores (one batch
image per core, params replicated).  Per core:
  - prologue: fm via small matmuls (W1 scattered into a [128,16] lhsT) +
    relu (scalar engine); fm is round-tripped to DRAM (per 1024-pixel group)
    because DMA partition-broadcast needs a DRAM source.
  - fm row g is broadcast to 128 partitions in [128,1024] chunks, split
    between DRAM->SBUF broadcast DMAs (wide shapes fan out across all 16 DMA
    engines) and gpsimd.partition_broadcast, so no compute engine pays for
    replication.
  - feat = x * fm_rep on the vector engine as a pure-SBUF bf16 multiply
    (2x mode, ~430ns per [128,512]).
  - the PE runs only the 256 accumulating K=128 main matmuls (plus 8 fm
    matmuls), all bf16 N=512.
  Keeping the vector engine away from PSUM and the PE free of K-switches is
  what lets the PE run at its warm 2.4 GHz rate (a DVE op reading PSUM every
  iteration was measured to hold the PE at half clock for the whole kernel).
Accuracy: bf16 matmuls with fp32 PSUM accumulation; rel err ~4e-3.
"""

import numpy as np
import ml_dtypes

import concourse.bacc as bacc
import concourse.tile as tile
from concourse import mybir
from concourse.bass_utils import run_bass_kernel_spmd

F32 = mybir.dt.float32
BF16 = mybir.dt.bfloat16

B, C, H, W = 8, 128, 64, 64
NUMS, HEADS, OUT = 16, 8, 256
P = H * W          # 4096 pixels per image
PB = 512           # pixel block (one PSUM bank of fp32)
NPB = P // PB      # 8 pixel blocks
GRP = 1024         # broadcast chunk (2 pixel blocks)
NGRP = P // GRP    # 4 broadcast groups
N_CORES = 8
LOOKAHEAD = 8      # broadcast/feat pipeline depth (in (g,k) units) ahead of mains
GPSIMD_GS = {2, 5, 8, 11, 14}      # groups replicated via gpsimd.partition_broadcast

_CACHE = {}


def _build():
    nc = bacc.Bacc("TRN2", target_bir_lowering=False, debug=False,
                   num_devices=N_CORES)

    x_d = nc.dram_tensor("x", [C, P], BF16, kind="ExternalInput")
    w1s_d = nc.dram_tensor("w1s", [C, NUMS], BF16, kind="ExternalInput")
    w2t_d = nc.dram_tensor("w2t", [C, NUMS * OUT], BF16, kind="ExternalInput")
    b1_d = nc.dram_tensor("b1c", [NUMS, 1], F32, kind="ExternalInput")
    b2_d = nc.dram_tensor("b2c", [C, 2], F32, kind="ExternalInput")
    out_d = nc.dram_tensor("out", [OUT, P], F32, kind="ExternalOutput")

    relu = mybir.ActivationFunctionType.Relu
    ident = mybir.ActivationFunctionType.Identity
    mult = mybir.AluOpType.mult

    with tile.TileContext(nc) as tc:
        with (
            tc.tile_pool(name="const", bufs=1) as cpool,
            tc.tile_pool(name="xp", bufs=2) as xp,
            tc.tile_pool(name="xbp", bufs=1) as xbp,
            tc.tile_pool(name="fmrow", bufs=1) as fmrowp,
            tc.tile_pool(name="repp", bufs=20) as repp,
            tc.tile_pool(name="feat", bufs=2 * LOOKAHEAD + 2) as featp,
            tc.tile_pool(name="osb", bufs=4) as osb,
            tc.tile_pool(name="ps", bufs=8, space="PSUM") as ps,
            tc.tile_pool(name="dr", bufs=4, space="DRAM") as drp,
        ):
            # ---- prologue: x load/cast + fm, per broadcast group ----
            w1s_t = cpool.tile([C, NUMS], BF16)
            nc.sync.dma_start(w1s_t[:], w1s_d[:])
            b1_t = cpool.tile([NUMS, 1], F32)
            nc.sync.dma_start(b1_t[:], b1_d[:])

            fm_sb = cpool.tile([NUMS, P], BF16)
            fm_drs = [drp.tile([NUMS, GRP], BF16, tag=f"fmdr{k}",
                               name=f"fmdr{k}")
                      for k in range(NGRP)]
            fmrows = {}
            x2s = [None] * NGRP

            def emit_fm_write(k):
                gx = slice(k * GRP, (k + 1) * GRP)
                nc.sync.dma_start(fm_drs[k][:], fm_sb[:, gx])
                for g in sorted(GPSIMD_GS):
                    fr = fmrowp.tile([1, GRP], BF16, tag=f"fr{g}_{k}",
                                     name=f"fr{g}_{k}")
                    nc.scalar.dma_start(fr[:], fm_drs[k][g:g + 1, :])
                    fmrows[(g, k)] = fr

            for k in range(NGRP):
                x2 = xbp.tile([C, GRP], BF16, tag=f"x2_{k}", name=f"x2_{k}")
                x2s[k] = x2
                gx = slice(k * GRP, (k + 1) * GRP)
                nc.sync.dma_start(x2[:], x_d[:, gx])
                for half in range(2):
                    pb = 2 * k + half
                    px = slice(pb * PB, (pb + 1) * PB)
                    hx = slice(half * PB, (half + 1) * PB)
                    ps_fm = ps.tile([NUMS, PB], F32, tag="ps",
                                    name=f"psfm{pb}")
                    nc.tensor.matmul(ps_fm[:], w1s_t[:], x2[:, hx],
                                     start=True, stop=True)
                    nc.scalar.activation(fm_sb[:, px], ps_fm[:], relu,
                                         bias=b1_t[:])
                if k == 0:
                    emit_fm_write(0)
                    w2t_t = cpool.tile([C, NUMS * OUT], BF16)
                    nc.sync.dma_start(w2t_t[:], w2t_d[:])
                    b2_t = cpool.tile([C, 2], F32)
                    nc.sync.dma_start(b2_t[:], b2_d[:])

            # ---- replication + feat, pipelined ahead of the mains ----
            nbc = [0]

            def emit_rep_grp(g, k):
                rep = repp.tile([C, GRP], BF16, tag="rep", name=f"rep{g}_{k}")
                if g in GPSIMD_GS:
                    nc.gpsimd.partition_broadcast(rep[:], fmrows[(g, k)][0:1, :])
                else:
                    eng = nc.sync if nbc[0] % 2 == 0 else nc.scalar
                    nbc[0] += 1
                    eng.dma_start(rep[:],
                                  fm_drs[k][g:g + 1, :].broadcast_to((C, GRP)))
                return rep

            fts = {}      # (g, k) -> [C, GRP] feat tile

            def emit_ft(g, k):
                rep = emit_rep_grp(g, k)
                ft = featp.tile([C, GRP], BF16, tag="ft", name=f"ft{g}_{k}")
                nc.vector.tensor_tensor(ft[:], x2s[k][:], rep[:], op=mult)
                fts[(g, k)] = ft

            todo = [(g, k) for k in range(NGRP) for g in range(NUMS)]
            for i in range(LOOKAHEAD):
                emit_ft(*todo[i])

            pso = {}
            for i, (g, k) in enumerate(todo):
                if g == 0 and k + 1 < NGRP:
                    emit_fm_write(k + 1)
                if i + LOOKAHEAD < len(todo):
                    emit_ft(*todo[i + LOOKAHEAD])
                ft = fts.pop((g, k))
                if g == 0:
                    for pbb in (2 * k, 2 * k + 1):
                        for oc in range(2):
                            t = ps.tile([C, PB], F32, tag="ps",
                                        name=f"pso{pbb}_{oc}")
                            pso[(pbb, oc)] = t
                for half in range(2):
                    pb = 2 * k + half
                    hx = slice(half * PB, (half + 1) * PB)
                    nc.tensor.matmul(pso[(pb, 0)][:],
                                     w2t_t[:, (2 * g) * C:(2 * g + 1) * C],
                                     ft[:, hx], start=(g == 0),
                                     stop=(g == NUMS - 1))
                    nc.tensor.matmul(pso[(pb, 1)][:],
                                     w2t_t[:, (2 * g + 1) * C:(2 * g + 2) * C],
                                     ft[:, hx], start=(g == 0),
                                     stop=(g == NUMS - 1))
                if g == NUMS - 1:
                    for pbb in (2 * k, 2 * k + 1):
                        px = slice(pbb * PB, (pbb + 1) * PB)
                        o0 = osb.tile([C, PB], F32, tag="osb",
                                      name=f"o0_{pbb}")
                        o1 = osb.tile([C, PB], F32, tag="osb",
                                      name=f"o1_{pbb}")
                        nc.scalar.activation(o0[:], pso.pop((pbb, 0))[:],
                                             ident, bias=b2_t[:, 0:1])
                        nc.scalar.activation(o1[:], pso.pop((pbb, 1))[:],
                                             ident, bias=b2_t[:, 1:2])
                        nc.scalar.dma_start(out_d[0:C, px], o0[:])
                        nc.scalar.dma_start(out_d[C:OUT, px], o1[:])

    nc.compile()
    return nc


def _prep_params(W1, b1, W2, b2):
    bf = ml_dtypes.bfloat16
    # w1s[c, g] = W1[g, c - 8g] for 8g <= c < 8(g+1), else 0
    w1s = np.zeros((C, NUMS), dtype=bf)
    for g in range(NUMS):
        w1s[g * HEADS:(g + 1) * HEADS, g] = W1[g].astype(bf)
    # w2t[k, (g*2+oc)*128 + m] = W2[oc*128 + m, g*128 + k]
    w2t = (
        np.asarray(W2, dtype=np.float32)
        .reshape(2, C, NUMS, C)          # [oc, m, g, k]
        .transpose(3, 2, 0, 1)           # [k, g, oc, m]
        .reshape(C, NUMS * OUT)
        .astype(bf)
    )
    b1c = np.asarray(b1, dtype=np.float32).reshape(NUMS, 1).copy()
    b2c = np.asarray(b2, dtype=np.float32).reshape(2, C).T.copy()
    return w1s, w2t, b1c, b2c


def kernel(x, W1, b1, W2, b2, _trace=False, _trace_kwargs=None):
    if "nc" not in _CACHE:
        _CACHE["nc"] = _build()
    nc = _CACHE["nc"]

    w1s, w2t, b1c, b2c = _prep_params(W1, b1, W2, b2)
    xs = np.ascontiguousarray(
        np.asarray(x, dtype=np.float32).reshape(B, C, P).astype(ml_dtypes.bfloat16))
    in_maps = [
        {"x": xs[b_], "w1s": w1s, "w2t": w2t, "b1c": b1c, "b2c": b2c}
        for b_ in range(N_CORES)
    ]
    kwargs = {}
    if _trace:
        kwargs["trace"] = True
        kwargs.update(_trace_kwargs or {})
    res = run_bass_kernel_spmd(nc, in_maps, core_ids=list(range(N_CORES)),
                               **kwargs)
    out = np.stack([res.results[b_]["out"] for b_ in range(N_CORES)])
    out = out.reshape(B, OUT, H, W)
    if _trace:
        _CACHE["last_result"] = res
    return out
